# revision 33
# baseline (speedup 1.0000x reference)
"""Multi-head attention Trainium2 kernel (8 NeuronCores).

Problem: x[2,2048,1024] -> MHA(16 heads, d=64) -> out[2,2048,1024], fp32.

Sharding: 2-way data parallel on batch x 4-way tensor parallel on heads.
Core c handles batch c//4 and heads 4*(c%4) .. 4*(c%4)+3 (a 256-wide slice
of the Wq/Wk/Wv columns and Wo rows). Each core returns a partial output
[2048,1024] (bf16); the host sums the 4 TP partials per batch and adds the
bias terms (bo, and bv@Wo which is separable because softmax rows sum to 1;
bk drops out of softmax entirely since (q+bq)@bk is constant along keys).

All matmuls run in bf16 (1 cycle/row on the PE). Host pre-transposes and
pre-tiles every input so each DMA line is contiguous (>=2KB per partition
row). On-core dataflow per core:
  xt = x[b].T tiled [128, 8, 2048]      (DMA'd in 16 contiguous pieces)
  Q^T = Wq_g^T stationary over xt       [256, 2048]  (+bq, d on partitions)
  K^T likewise (no bias), V natural     [2048, 256]  via xt-stationary mms
  S^T[k,q] = K^T(d,k).T @ Q^T(d,q)      2 heads row-packed (d=64 each)
  P = exp(S^T / 32)                     ScalarE (exp only lives here)
  O'^T[d+1,q] = [V|ones].T @ P          ones column gives softmax denoms
  O^T = O'[0:64] * approx(1/denom)      DVE fast reciprocal + gpsimd bcast
  out = O^T.T @ Wo_g                    [2048, 1024] bf16 partial, DMA'd out

Schedule: attention is Scalar-bound (exp) and the projections/output matmul
are interleaved into the attention kc-loop as fillers so the PE never idles:
prologue K/Q for q<1024 of pair 0 only (DMA-paced), V + remaining K0/Q0 fill
call (0,0), K1/Q1 fill call (0,1), the first half of the output projection
fills call (1,1), and only out[q>=1024]'s projection trails the last call.
"""

import numpy as np

B = 2
N = 2048
E = 1024
HEADS = 16
D = 64
P = 128
NCORES = 8
GROUPS = 4            # TP groups
DG = E // GROUPS      # 256 cols per core
ECH = E // P          # 8 contraction chunks
NCH = N // P          # 16 sequence chunks
QS = 1024             # q span for softmax tiles
QB = 512              # matmul moving free dim

_CACHE = {}


def _build():
    import sys
    if "/opt/trn_rl_repo" not in sys.path:
        sys.path.insert(0, "/opt/trn_rl_repo")
    import concourse.tile as tile
    from concourse import bacc, mybir
    from concourse.bass import ts

    F32 = mybir.dt.float32
    BF16 = mybir.dt.bfloat16
    Exp = mybir.ActivationFunctionType.Exp

    nc = bacc.Bacc("TRN2", target_bir_lowering=False, debug=False, num_devices=NCORES)

    xt = nc.dram_tensor("xt", [P, ECH, N], BF16, kind="ExternalInput").ap()
    wq = nc.dram_tensor("wq", [P, ECH, DG], BF16, kind="ExternalInput").ap()
    wk = nc.dram_tensor("wk", [P, ECH, DG], BF16, kind="ExternalInput").ap()
    wv = nc.dram_tensor("wv", [P, ECH, DG], BF16, kind="ExternalInput").ap()
    wo = nc.dram_tensor("wo", [P, 2, E], BF16, kind="ExternalInput").ap()
    bq2 = nc.dram_tensor("bq2", [P, 2], F32, kind="ExternalInput").ap()
    out = nc.dram_tensor("out", [N, E], BF16, kind="ExternalOutput").ap()

    with tile.TileContext(nc) as tc:
        with tc.tile_pool(name="persist", bufs=1) as pers, \
             tc.tile_pool(name="pexp", bufs=12) as pexp_pool, \
             tc.tile_pool(name="small", bufs=2) as small, \
             tc.tile_pool(name="ostage", bufs=4) as ostage, \
             tc.tile_pool(name="ppmain", bufs=1, space="PSUM") as ppm, \
             tc.tile_pool(name="ppoacc", bufs=1, space="PSUM") as ppo:
            xt_sb = pers.tile([P, ECH, N], BF16, tag="xt")
            wq_sb = pers.tile([P, ECH, DG], BF16, tag="wq")
            wk_sb = pers.tile([P, ECH, DG], BF16, tag="wk")
            wv_sb = pers.tile([P, ECH, DG], BF16, tag="wv")
            wo_sb = pers.tile([P, 2, E], BF16, tag="wo")
            bq_sb = pers.tile([P, 2], F32, tag="bq")
            qT_p = [pers.tile([P, N], BF16, tag=f"qT{i}", name=f"qT{i}") for i in range(2)]
            kT_p = [pers.tile([P, N], BF16, tag=f"kT{i}", name=f"kT{i}") for i in range(2)]
            v_sb = pers.tile([P, NCH, GROUPS, 66], BF16, tag="v")
            oT_p = [pers.tile([P, N], BF16, tag=f"oT{i}", name=f"oT{i}") for i in range(2)]

            def proj_ps(i, name):
                return ppm.tile([P, QS], F32, tag="A" if i % 2 == 0 else "B", name=name)

            def qk_chain(pair, w_sb, dst, bias, qb):
                # one 512-wide q block of the Q^T/K^T projection: 8 ec-chunk
                # matmuls accumulated in one PSUM bank, then drained on DVE
                def emit():
                    ps = proj_ps(qb, f"qkps{pair}{qb}")
                    psl = ps[:, :QB]
                    for ec in range(ECH):
                        nc.tensor.matmul(
                            psl,
                            w_sb[:, ec, ts(pair, P)],
                            xt_sb[:, ec, ts(qb, QB)],
                            start=(ec == 0), stop=(ec == ECH - 1),
                        )
                    if bias:
                        nc.vector.tensor_add(
                            dst[:, ts(qb, QB)], psl,
                            bq_sb[:, pair, None].to_broadcast((P, QB)),
                        )
                    else:
                        nc.vector.tensor_copy(dst[:, ts(qb, QB)], psl)
                return emit

            def v_chain(ncx):
                def emit():
                    ps = proj_ps(ncx, f"vps{ncx}")
                    psl = ps[:, :DG]
                    for ec in range(ECH):
                        nc.tensor.matmul(
                            psl,
                            xt_sb[:, ec, ts(ncx, P)],
                            wv_sb[:, ec, :],
                            start=(ec == 0), stop=(ec == ECH - 1),
                        )
                    nc.vector.tensor_copy(
                        v_sb[:, ncx, :, 0:64],
                        psl.rearrange("p (h d) -> p h d", d=D),
                    )
                return emit

            def wo_chain(ncx, drain=None, dma_eng=None):
                # out[ncx*128:(ncx+1)*128, :]: both 512-wide halves of E share
                # each oT stationary load; accumulate over the two d-chunks
                def emit():
                    ps = ppm.tile([P, QS], F32, tag="A" if ncx % 2 == 0 else "B",
                                  name=f"wops{ncx}")
                    for dc in range(2):
                        for fb in range(2):
                            nc.tensor.matmul(
                                ps[:, ts(fb, QB)],
                                oT_p[dc][:, ts(ncx, P)],
                                wo_sb[:, dc, ts(fb, QB)],
                                start=(dc == 0), stop=(dc == 1),
                            )
                    ot = ostage.tile([P, QS], BF16, tag="ot", name="ot")
                    (drain or nc.vector.tensor_copy)(ot, ps)
                    (dma_eng or nc.sync).dma_start(out[ts(ncx, P), :], ot)
                return emit

            def emit_attn(pair, qs, fills=(), finish_prev=None, finish_kc=2):
                # fills: dict kc -> list of filler emitters.  finish_prev:
                # the previous call's deferred normalize tail — emitted at a
                # kc whose neighborhood has no fill drains, so those drains
                # (which hand PSUM banks back to the S matmuls) never queue
                # behind it on the in-order DVE.  Returns this call's own
                # deferred tail.
                fills = dict(fills)
                if finish_prev is not None:
                    fills.setdefault(finish_kc, []).append(finish_prev)
                oaccs = [ppo.tile([65, QS], F32, tag=f"O{h}", name=f"oacc{h}")
                         for h in range(2)]
                def emit_pv(kc, pes):
                    for h in range(2):
                        hh = 2 * pair + h
                        for qb in range(QS // QB):
                            nc.tensor.matmul(
                                oaccs[h][:, ts(qb, QB)],
                                v_sb[:, kc, hh, 0:65],
                                pes[h][:, ts(qb, QB)],
                                start=(kc == 0), stop=(kc == NCH - 1),
                            )

                # software-pipelined: kc's PVs are emitted after kc+1's S
                # matmuls so every PE instruction is ready when the in-order
                # engine reaches it — no wait-queue parking, no PE idle, and
                # the clock stays at the full p-state
                prev = None
                for kc in range(NCH):
                    for f in fills.pop(kc, ()):
                        f()
                    pes = []
                    for h in range(2):
                        # qb-inner: both q halves share the kT stationary, and
                        # exp(h) is dispatchable right after its own S pair
                        ps = ppm.tile([P, QS], F32, tag="AB"[h], name=f"spsum{h}")
                        psl = slice(D * h, D * h + D)
                        for qb in range(QS // QB):
                            nc.tensor.matmul(
                                ps[:, ts(qb, QB)],
                                kT_p[pair][psl, ts(kc, P)],
                                qT_p[pair][psl, qs * QS + qb * QB:qs * QS + (qb + 1) * QB],
                                start=True, stop=True,
                            )
                        pe = pexp_pool.tile([P, QS], BF16, tag="pexp", name="pexp")
                        nc.scalar.activation(pe, ps, Exp, scale=1.0 / 32.0)
                        pes.append(pe)
                    if prev is not None:
                        emit_pv(*prev)
                    prev = (kc, pes)
                emit_pv(*prev)
                for kc, fl in sorted(fills.items()):
                    for f in fl:
                        f()
                # free the oacc PSUM banks right away (the next call's first
                # PVs reuse them); the rest of the normalize is deferred
                osps = []
                for h in range(2):
                    osp = small.tile([65, QS], F32, tag=f"osp{h}", name="osp", bufs=1)
                    nc.vector.tensor_copy(osp, oaccs[h])
                    osps.append(osp)

                def finish(half=None):
                    # half=None: full QS width; half=0/1: one 512 sub-span
                    # (lets the tail start its wo_chains after half the work)
                    w = QS if half is None else QB
                    f0 = 0 if half is None else half * QB
                    for h in range(2):
                        psl = slice(D * h, D * h + D)
                        d2 = small.tile([1, w], F32, tag=f"d2{h}{half}", name="d2", bufs=1)
                        nc.vector.tensor_copy(d2, osps[h][64:65, f0:f0 + w])
                        r2 = small.tile([1, w], F32, tag=f"r2{h}{half}", name="r2", bufs=1)
                        nc.vector.reciprocal_approx_fast(r2, d2)
                        rbc = small.tile([P, w], F32, tag=f"rbc{half}", name="rbc")
                        nc.gpsimd.partition_broadcast(rbc, r2)
                        nc.vector.tensor_mul(
                            oT_p[pair][psl, qs * QS + f0:qs * QS + f0 + w],
                            osps[h][0:64, f0:f0 + w],
                            rbc[0:64, :],
                        )
                return finish

            # --- DMA: one queue, ordered by first use — transfers only begin
            # ~8.4us in (runtime startup) and stripe across all engines at
            # ~400GB/s, so the queue order IS the arrival order
            nc.sync.dma_start(wk_sb, wk)
            nc.sync.dma_start(xt_sb[:, ts(0, 4), ts(0, QS)], xt[:, ts(0, 4), ts(0, QS)])
            nc.sync.dma_start(wq_sb, wq)
            nc.sync.dma_start(xt_sb[:, ts(1, 4), ts(0, QS)], xt[:, ts(1, 4), ts(0, QS)])
            nc.sync.dma_start(wv_sb, wv)
            nc.sync.dma_start(bq_sb, bq2)
            nc.sync.dma_start(xt_sb[:, ts(0, 4), ts(1, QS)], xt[:, ts(0, 4), ts(1, QS)])
            nc.sync.dma_start(xt_sb[:, ts(1, 4), ts(1, QS)], xt[:, ts(1, 4), ts(1, QS)])
            nc.sync.dma_start(wo_sb, wo)

            ones_f32 = pers.tile([P, 1], F32, tag="ones")
            nc.vector.memset(ones_f32, 1.0)
            nc.vector.tensor_copy(
                v_sb[:, :, :, 64:65],
                ones_f32[:, 0, None, None, None].to_broadcast((P, NCH, GROUPS, 1)),
            )

            # prologue: K0/Q0 for q<1024 only (paced by the xt nh0 DMAs)
            for qb in range(2):
                qk_chain(0, wk_sb, kT_p[0], False, qb)()
            for qb in range(2):
                qk_chain(0, wq_sb, qT_p[0], True, qb)()

            # call (0,0): V paced 2/kc from kc1 (wv/xt surely landed — a
            # not-yet-ready fill would clog the 4-deep PE wait queue), then
            # the q>=1024 half of K0/Q0 (needed from call (0,1) kc0)
            fills00 = {kc + 1: [v_chain(2 * kc), v_chain(2 * kc + 1)] for kc in range(8)}
            # K-qb2/qb3 feed this very call's S at kc8/kc12 (kT cols are the
            # key chunks); Q-qb2/qb3 only feed call (0,1)
            fills00[8].append(qk_chain(0, wk_sb, kT_p[0], False, 2))
            fills00[9] = [qk_chain(0, wk_sb, kT_p[0], False, 3)]
            fills00[10] = [qk_chain(0, wq_sb, qT_p[0], True, 2)]
            fills00[11] = [qk_chain(0, wq_sb, qT_p[0], True, 3)]
            fin = emit_attn(0, 0, fills00)

            # call (0,1): K1/Q1 fillers, front-loaded (needed by calls (1,*))
            fills01 = {}
            for qb in range(4):
                fills01[2 * qb] = [qk_chain(1, wk_sb, kT_p[1], False, qb)]
                fills01[2 * qb + 1] = [qk_chain(1, wq_sb, qT_p[1], True, qb)]
            fin = emit_attn(0, 1, fills01, finish_prev=fin)

            fin = emit_attn(1, 0, finish_prev=fin)

            # call (1,1): output projection for q<1024, in the back half so
            # the (1,0) normalize (oT1 writes, deferred to kc2) has drained
            fills11 = {8 + i: [wo_chain(i)] for i in range(8)}
            fin = emit_attn(1, 1, fills11, finish_prev=fin)
            fin()

            # tail: output projection for q>=1024; stage copies on the (now
            # idle) Scalar engine so they don't queue behind the last
            # normalize on DVE, and alternate output DMAs across the SP and
            # Activation queues so the final drain runs at 2x
            for ncx in range(8, 16):
                wo_chain(ncx, drain=nc.scalar.copy,
                         dma_eng=nc.scalar if ncx % 2 else nc.sync)()

    nc.compile()
    return nc


def _get_nc():
    if "nc" not in _CACHE:
        _CACHE["nc"] = _build()
    return _CACHE["nc"]


def kernel(x, Wq, bq, Wk, bk, Wv, bv, Wo, bo, **run_kwargs):
    import sys
    if "/opt/trn_rl_repo" not in sys.path:
        sys.path.insert(0, "/opt/trn_rl_repo")
    import ml_dtypes
    from concourse.bass_utils import run_bass_kernel_spmd

    BF = ml_dtypes.bfloat16
    x = np.asarray(x, dtype=np.float32)
    Wq = np.asarray(Wq, dtype=np.float32)
    Wk = np.asarray(Wk, dtype=np.float32)
    Wv = np.asarray(Wv, dtype=np.float32)
    Wo = np.asarray(Wo, dtype=np.float32)
    bq = np.asarray(bq, dtype=np.float32)
    bv = np.asarray(bv, dtype=np.float32)
    bo = np.asarray(bo, dtype=np.float32)

    nc = _get_nc()

    def tile_rows(a, d0):
        # [d0*P, M] -> [P, d0, M]
        return np.ascontiguousarray(
            a.reshape(d0, P, -1).transpose(1, 0, 2)).astype(BF)

    in_maps = []
    xts = [tile_rows(np.ascontiguousarray(x[b].T), ECH) for b in range(B)]
    for c in range(NCORES):
        b, g = divmod(c, GROUPS)
        cols = slice(g * DG, (g + 1) * DG)
        in_maps.append({
            "xt": xts[b],
            "wq": tile_rows(Wq[:, cols], ECH),
            "wk": tile_rows(Wk[:, cols], ECH),
            "wv": tile_rows(Wv[:, cols], ECH),
            "wo": tile_rows(Wo[cols, :], 2),
            "bq2": np.ascontiguousarray(bq[cols].reshape(2, P).T),
        })

    try:
        res = run_bass_kernel_spmd(nc, in_maps, core_ids=list(range(NCORES)), **run_kwargs)
    except Exception:
        # device may be wedged from a prior run; reset the accelerator once
        try:
            import ctypes
            lib = ctypes.CDLL("/opt/axon/libaxon_pjrt.so")
            lib.axon_reset.restype = ctypes.c_int
            lib.axon_reset()
        except Exception:
            pass
        res = run_bass_kernel_spmd(nc, in_maps, core_ids=list(range(NCORES)), **run_kwargs)
    if run_kwargs:
        _CACHE["last_results"] = res

    # gather: sum TP partials per batch, add separable bias terms
    bias_vec = bv @ Wo + bo  # softmax rows sum to 1 => bv contributes bv@Wo
    full = np.empty((B, N, E), dtype=np.float32)
    for b in range(B):
        acc = res.results[b * GROUPS]["out"].astype(np.float32)
        for g in range(1, GROUPS):
            acc = acc + res.results[b * GROUPS + g]["out"].astype(np.float32)
        full[b] = acc + bias_vec[None, :]
    return full


# revision 35
# speedup vs baseline: 1.0092x; 1.0092x over previous
"""Multi-head attention Trainium2 kernel (8 NeuronCores).

Problem: x[2,2048,1024] -> MHA(16 heads, d=64) -> out[2,2048,1024], fp32.

Sharding: 2-way data parallel on batch x 4-way tensor parallel on heads.
Core c handles batch c//4 and heads 4*(c%4) .. 4*(c%4)+3 (a 256-wide slice
of the Wq/Wk/Wv columns and Wo rows). Each core returns a partial output
[2048,1024] (bf16); the host sums the 4 TP partials per batch and adds the
bias terms (bo, and bv@Wo which is separable because softmax rows sum to 1;
bk drops out of softmax entirely since (q+bq)@bk is constant along keys).

All matmuls run in bf16 (1 cycle/row on the PE). Host pre-transposes and
pre-tiles every input so each DMA line is contiguous (>=2KB per partition
row). On-core dataflow per core:
  xt = x[b].T tiled [128, 8, 2048]      (DMA'd in 16 contiguous pieces)
  Q^T = Wq_g^T stationary over xt       [256, 2048]  (+bq, d on partitions)
  K^T likewise (no bias), V natural     [2048, 256]  via xt-stationary mms
  S^T[k,q] = K^T(d,k).T @ Q^T(d,q)      2 heads row-packed (d=64 each)
  P = exp(S^T / 32)                     ScalarE (exp only lives here)
  O'^T[d+1,q] = [V|ones].T @ P          ones column gives softmax denoms
  O^T = O'[0:64] * approx(1/denom)      DVE fast reciprocal + gpsimd bcast
  out = O^T.T @ Wo_g                    [2048, 1024] bf16 partial, DMA'd out

Schedule: attention is Scalar-bound (exp) and the projections/output matmul
are interleaved into the attention kc-loop as fillers so the PE never idles:
prologue K/Q for q<1024 of pair 0 only (DMA-paced), V + remaining K0/Q0 fill
call (0,0), K1/Q1 fill call (0,1), the first half of the output projection
fills call (1,1), and only out[q>=1024]'s projection trails the last call.
"""

import numpy as np

B = 2
N = 2048
E = 1024
HEADS = 16
D = 64
P = 128
NCORES = 8
GROUPS = 4            # TP groups
DG = E // GROUPS      # 256 cols per core
ECH = E // P          # 8 contraction chunks
NCH = N // P          # 16 sequence chunks
QS = 1024             # q span for softmax tiles
QB = 512              # matmul moving free dim

_CACHE = {}


def _build():
    import sys
    if "/opt/trn_rl_repo" not in sys.path:
        sys.path.insert(0, "/opt/trn_rl_repo")
    import concourse.tile as tile
    from concourse import bacc, mybir
    from concourse.bass import ts

    F32 = mybir.dt.float32
    BF16 = mybir.dt.bfloat16
    Exp = mybir.ActivationFunctionType.Exp

    nc = bacc.Bacc("TRN2", target_bir_lowering=False, debug=False, num_devices=NCORES)

    xt = nc.dram_tensor("xt", [P, ECH, N], BF16, kind="ExternalInput").ap()
    wq = nc.dram_tensor("wq", [P, ECH, DG], BF16, kind="ExternalInput").ap()
    wk = nc.dram_tensor("wk", [P, ECH, DG], BF16, kind="ExternalInput").ap()
    wv = nc.dram_tensor("wv", [P, ECH, DG], BF16, kind="ExternalInput").ap()
    wo = nc.dram_tensor("wo", [P, 2, E], BF16, kind="ExternalInput").ap()
    bq2 = nc.dram_tensor("bq2", [P, 2], F32, kind="ExternalInput").ap()
    out = nc.dram_tensor("out", [N, E], BF16, kind="ExternalOutput").ap()

    with tile.TileContext(nc) as tc:
        with tc.tile_pool(name="persist", bufs=1) as pers, \
             tc.tile_pool(name="pexp", bufs=12) as pexp_pool, \
             tc.tile_pool(name="small", bufs=2) as small, \
             tc.tile_pool(name="ostage", bufs=4) as ostage, \
             tc.tile_pool(name="ppmain", bufs=1, space="PSUM") as ppm, \
             tc.tile_pool(name="ppoacc", bufs=1, space="PSUM") as ppo:
            xt_sb = pers.tile([P, ECH, N], BF16, tag="xt")
            wq_sb = pers.tile([P, ECH, DG], BF16, tag="wq")
            wk_sb = pers.tile([P, ECH, DG], BF16, tag="wk")
            wv_sb = pers.tile([P, ECH, DG], BF16, tag="wv")
            wo_sb = pers.tile([P, 2, E], BF16, tag="wo")
            bq_sb = pers.tile([P, 2], F32, tag="bq")
            qT_p = [pers.tile([P, N], BF16, tag=f"qT{i}", name=f"qT{i}") for i in range(2)]
            kT_p = [pers.tile([P, N], BF16, tag=f"kT{i}", name=f"kT{i}") for i in range(2)]
            v_sb = pers.tile([P, NCH, GROUPS, 66], BF16, tag="v")
            oT_p = [pers.tile([P, N], BF16, tag=f"oT{i}", name=f"oT{i}") for i in range(2)]

            def proj_ps(i, name):
                return ppm.tile([P, QS], F32, tag="A" if i % 2 == 0 else "B", name=name)

            def qk_chain(pair, w_sb, dst, bias, qb):
                # one 512-wide q block of the Q^T/K^T projection: 8 ec-chunk
                # matmuls accumulated in one PSUM bank, then drained on DVE
                def emit():
                    ps = proj_ps(qb, f"qkps{pair}{qb}")
                    psl = ps[:, :QB]
                    for ec in range(ECH):
                        nc.tensor.matmul(
                            psl,
                            w_sb[:, ec, ts(pair, P)],
                            xt_sb[:, ec, ts(qb, QB)],
                            start=(ec == 0), stop=(ec == ECH - 1),
                        )
                    if bias:
                        nc.vector.tensor_add(
                            dst[:, ts(qb, QB)], psl,
                            bq_sb[:, pair, None].to_broadcast((P, QB)),
                        )
                    else:
                        nc.vector.tensor_copy(dst[:, ts(qb, QB)], psl)
                return emit

            def v_chain(ncx):
                def emit():
                    ps = proj_ps(ncx, f"vps{ncx}")
                    psl = ps[:, :DG]
                    for ec in range(ECH):
                        nc.tensor.matmul(
                            psl,
                            xt_sb[:, ec, ts(ncx, P)],
                            wv_sb[:, ec, :],
                            start=(ec == 0), stop=(ec == ECH - 1),
                        )
                    nc.vector.tensor_copy(
                        v_sb[:, ncx, :, 0:64],
                        psl.rearrange("p (h d) -> p h d", d=D),
                    )
                return emit

            def wo_chain(ncx, drain=None, dma_eng=None):
                # out[ncx*128:(ncx+1)*128, :]: both 512-wide halves of E share
                # each oT stationary load; accumulate over the two d-chunks
                def emit():
                    ps = ppm.tile([P, QS], F32, tag="A" if ncx % 2 == 0 else "B",
                                  name=f"wops{ncx}")
                    for dc in range(2):
                        for fb in range(2):
                            nc.tensor.matmul(
                                ps[:, ts(fb, QB)],
                                oT_p[dc][:, ts(ncx, P)],
                                wo_sb[:, dc, ts(fb, QB)],
                                start=(dc == 0), stop=(dc == 1),
                            )
                    ot = ostage.tile([P, QS], BF16, tag="ot", name="ot")
                    (drain or nc.vector.tensor_copy)(ot, ps)
                    (dma_eng or nc.sync).dma_start(out[ts(ncx, P), :], ot)
                return emit

            def emit_attn(pair, qs, fills=(), finish_prev=None, finish_kc=2):
                # fills: dict kc -> list of filler emitters.  finish_prev:
                # the previous call's deferred normalize tail — emitted at a
                # kc whose neighborhood has no fill drains, so those drains
                # (which hand PSUM banks back to the S matmuls) never queue
                # behind it on the in-order DVE.  Returns this call's own
                # deferred tail.
                fills = dict(fills)
                if finish_prev is not None:
                    fills.setdefault(finish_kc, []).insert(0, finish_prev)
                oaccs = [ppo.tile([65, QS], F32, tag=f"O{h}", name=f"oacc{h}")
                         for h in range(2)]
                def emit_pv(kc, pes):
                    for h in range(2):
                        hh = 2 * pair + h
                        for qb in range(QS // QB):
                            nc.tensor.matmul(
                                oaccs[h][:, ts(qb, QB)],
                                v_sb[:, kc, hh, 0:65],
                                pes[h][:, ts(qb, QB)],
                                start=(kc == 0), stop=(kc == NCH - 1),
                            )

                # software-pipelined: kc's PVs are emitted after kc+1's S
                # matmuls so every PE instruction is ready when the in-order
                # engine reaches it — no wait-queue parking, no PE idle, and
                # the clock stays at the full p-state
                prev = None
                for kc in range(NCH):
                    for f in fills.pop(kc, ()):
                        f()
                    pes = []
                    for h in range(2):
                        # qb-inner: both q halves share the kT stationary, and
                        # exp(h) is dispatchable right after its own S pair
                        ps = ppm.tile([P, QS], F32, tag="AB"[h], name=f"spsum{h}")
                        psl = slice(D * h, D * h + D)
                        for qb in range(QS // QB):
                            nc.tensor.matmul(
                                ps[:, ts(qb, QB)],
                                kT_p[pair][psl, ts(kc, P)],
                                qT_p[pair][psl, qs * QS + qb * QB:qs * QS + (qb + 1) * QB],
                                start=True, stop=True,
                            )
                        pe = pexp_pool.tile([P, QS], BF16, tag="pexp", name="pexp")
                        nc.scalar.activation(pe, ps, Exp, scale=1.0 / 32.0)
                        pes.append(pe)
                    if prev is not None:
                        emit_pv(*prev)
                    prev = (kc, pes)
                emit_pv(*prev)
                for kc, fl in sorted(fills.items()):
                    for f in fl:
                        f()
                # free the oacc PSUM banks right away (the next call's first
                # PVs reuse them); the rest of the normalize is deferred
                osps = []
                for h in range(2):
                    osp = small.tile([65, QS], F32, tag=f"osp{h}", name="osp", bufs=1)
                    nc.vector.tensor_copy(osp, oaccs[h])
                    osps.append(osp)

                def finish():
                    for h in range(2):
                        psl = slice(D * h, D * h + D)
                        d2 = small.tile([1, QS], F32, tag=f"d2{h}", name="d2", bufs=1)
                        nc.vector.tensor_copy(d2, osps[h][64:65, :])
                        r2 = small.tile([1, QS], F32, tag=f"r2{h}", name="r2", bufs=1)
                        nc.vector.reciprocal_approx_fast(r2, d2)
                        rbc = small.tile([P, QS], F32, tag="rbc", name="rbc")
                        nc.gpsimd.partition_broadcast(rbc, r2)
                        nc.vector.tensor_mul(
                            oT_p[pair][psl, ts(qs, QS)],
                            osps[h][0:64, :],
                            rbc[0:64, :],
                        )
                return finish

            # --- DMA: one queue, ordered by first use — transfers only begin
            # ~8.4us in (runtime startup) and stripe across all engines at
            # ~400GB/s, so the queue order IS the arrival order
            nc.sync.dma_start(wk_sb, wk)
            nc.sync.dma_start(xt_sb[:, ts(0, 4), ts(0, QS)], xt[:, ts(0, 4), ts(0, QS)])
            nc.sync.dma_start(wq_sb, wq)
            nc.sync.dma_start(xt_sb[:, ts(1, 4), ts(0, QS)], xt[:, ts(1, 4), ts(0, QS)])
            nc.sync.dma_start(wv_sb, wv)
            nc.sync.dma_start(bq_sb, bq2)
            nc.sync.dma_start(xt_sb[:, ts(0, 4), ts(1, QS)], xt[:, ts(0, 4), ts(1, QS)])
            nc.sync.dma_start(xt_sb[:, ts(1, 4), ts(1, QS)], xt[:, ts(1, 4), ts(1, QS)])
            nc.sync.dma_start(wo_sb, wo)

            ones_f32 = pers.tile([P, 1], F32, tag="ones")
            nc.vector.memset(ones_f32, 1.0)
            nc.vector.tensor_copy(
                v_sb[:, :, :, 64:65],
                ones_f32[:, 0, None, None, None].to_broadcast((P, NCH, GROUPS, 1)),
            )

            # prologue: K0/Q0 for q<1024 only (paced by the xt nh0 DMAs)
            for qb in range(2):
                qk_chain(0, wk_sb, kT_p[0], False, qb)()
            for qb in range(2):
                qk_chain(0, wq_sb, qT_p[0], True, qb)()

            # call (0,0): V paced 2/kc from kc1 (wv/xt surely landed — a
            # not-yet-ready fill would clog the 4-deep PE wait queue), then
            # the q>=1024 half of K0/Q0 (needed from call (0,1) kc0)
            fills00 = {kc + 1: [v_chain(2 * kc), v_chain(2 * kc + 1)] for kc in range(8)}
            # K-qb2/qb3 feed this very call's S at kc8/kc12 (kT cols are the
            # key chunks); Q-qb2/qb3 only feed call (0,1)
            fills00[8].append(qk_chain(0, wk_sb, kT_p[0], False, 2))
            fills00[9] = [qk_chain(0, wk_sb, kT_p[0], False, 3)]
            fills00[10] = [qk_chain(0, wq_sb, qT_p[0], True, 2)]
            fills00[11] = [qk_chain(0, wq_sb, qT_p[0], True, 3)]
            fin = emit_attn(0, 0, fills00)

            # call (0,1): K1/Q1 fillers, front-loaded (needed by calls (1,*))
            fills01 = {}
            for qb in range(4):
                fills01[2 * qb] = [qk_chain(1, wk_sb, kT_p[1], False, qb)]
                fills01[2 * qb + 1] = [qk_chain(1, wq_sb, qT_p[1], True, qb)]
            fin = emit_attn(0, 1, fills01, finish_prev=fin)

            fin = emit_attn(1, 0, finish_prev=fin)

            # call (1,1): output projection for q<1024, in the back half so
            # the (1,0) normalize (oT1 writes, deferred to kc2) has drained
            fills11 = {8 + i: [wo_chain(i)] for i in range(8)}
            fin = emit_attn(1, 1, fills11, finish_prev=fin)
            fin()

            # tail: output projection for q>=1024; stage copies on the (now
            # idle) Scalar engine so they don't queue behind the last
            # normalize on DVE, and alternate output DMAs across the SP and
            # Activation queues so the final drain runs at 2x
            for ncx in range(8, 16):
                wo_chain(ncx, drain=nc.scalar.copy,
                         dma_eng=nc.scalar if ncx % 2 else nc.sync)()

    nc.compile()
    return nc


def _get_nc():
    if "nc" not in _CACHE:
        _CACHE["nc"] = _build()
    return _CACHE["nc"]


def kernel(x, Wq, bq, Wk, bk, Wv, bv, Wo, bo, **run_kwargs):
    import sys
    if "/opt/trn_rl_repo" not in sys.path:
        sys.path.insert(0, "/opt/trn_rl_repo")
    import ml_dtypes
    from concourse.bass_utils import run_bass_kernel_spmd

    BF = ml_dtypes.bfloat16
    x = np.asarray(x, dtype=np.float32)
    Wq = np.asarray(Wq, dtype=np.float32)
    Wk = np.asarray(Wk, dtype=np.float32)
    Wv = np.asarray(Wv, dtype=np.float32)
    Wo = np.asarray(Wo, dtype=np.float32)
    bq = np.asarray(bq, dtype=np.float32)
    bv = np.asarray(bv, dtype=np.float32)
    bo = np.asarray(bo, dtype=np.float32)

    nc = _get_nc()

    def tile_rows(a, d0):
        # [d0*P, M] -> [P, d0, M]
        return np.ascontiguousarray(
            a.reshape(d0, P, -1).transpose(1, 0, 2)).astype(BF)

    in_maps = []
    xts = [tile_rows(np.ascontiguousarray(x[b].T), ECH) for b in range(B)]
    for c in range(NCORES):
        b, g = divmod(c, GROUPS)
        cols = slice(g * DG, (g + 1) * DG)
        in_maps.append({
            "xt": xts[b],
            "wq": tile_rows(Wq[:, cols], ECH),
            "wk": tile_rows(Wk[:, cols], ECH),
            "wv": tile_rows(Wv[:, cols], ECH),
            "wo": tile_rows(Wo[cols, :], 2),
            "bq2": np.ascontiguousarray(bq[cols].reshape(2, P).T),
        })

    try:
        res = run_bass_kernel_spmd(nc, in_maps, core_ids=list(range(NCORES)), **run_kwargs)
    except Exception:
        # device may be wedged from a prior run; reset the accelerator once
        try:
            import ctypes
            lib = ctypes.CDLL("/opt/axon/libaxon_pjrt.so")
            lib.axon_reset.restype = ctypes.c_int
            lib.axon_reset()
        except Exception:
            pass
        res = run_bass_kernel_spmd(nc, in_maps, core_ids=list(range(NCORES)), **run_kwargs)
    if run_kwargs:
        _CACHE["last_results"] = res

    # gather: sum TP partials per batch, add separable bias terms
    bias_vec = bv @ Wo + bo  # softmax rows sum to 1 => bv contributes bv@Wo
    full = np.empty((B, N, E), dtype=np.float32)
    for b in range(B):
        acc = res.results[b * GROUPS]["out"].astype(np.float32)
        for g in range(1, GROUPS):
            acc = acc + res.results[b * GROUPS + g]["out"].astype(np.float32)
        full[b] = acc + bias_vec[None, :]
    return full


# revision 36
# speedup vs baseline: 1.1867x; 1.1759x over previous
"""Multi-head attention Trainium2 kernel (8 NeuronCores).

Problem: x[2,2048,1024] -> MHA(16 heads, d=64) -> out[2,2048,1024], fp32.

Sharding: 2-way data parallel on batch x 4-way tensor parallel on heads.
Core c handles batch c//4 and heads 4*(c%4) .. 4*(c%4)+3 (a 256-wide slice
of the Wq/Wk/Wv columns and Wo rows). Each core returns a partial output
[2048,1024] (bf16); the host sums the 4 TP partials per batch and adds the
bias terms (bo, and bv@Wo which is separable because softmax rows sum to 1;
bk drops out of softmax entirely since (q+bq)@bk is constant along keys).

All matmuls run in bf16 (1 cycle/row on the PE). Host pre-transposes and
pre-tiles every input so each DMA line is contiguous (>=2KB per partition
row). On-core dataflow per core:
  xt = x[b].T tiled [128, 8, 2048]      (DMA'd in 16 contiguous pieces)
  Q^T = Wq_g^T stationary over xt       [256, 2048]  (+bq, d on partitions)
  K^T likewise (no bias), V natural     [2048, 256]  via xt-stationary mms
  S^T[k,q] = K^T(d,k).T @ Q^T(d,q)      2 heads row-packed (d=64 each)
  P = exp(S^T / 32)                     ScalarE (exp only lives here)
  O'^T[d+1,q] = [V|ones].T @ P          ones column gives softmax denoms
  O^T = O'[0:64] * approx(1/denom)      DVE fast reciprocal + gpsimd bcast
  out = O^T.T @ Wo_g                    [2048, 1024] bf16 partial, DMA'd out

Schedule: attention is Scalar-bound (exp) and the projections/output matmul
are interleaved into the attention kc-loop as fillers so the PE never idles:
prologue K/Q for q<1024 of pair 0 only (DMA-paced), V + remaining K0/Q0 fill
call (0,0), K1/Q1 fill call (0,1), the first half of the output projection
fills call (1,1), and only out[q>=1024]'s projection trails the last call.
"""

import numpy as np

B = 2
N = 2048
E = 1024
HEADS = 16
D = 64
P = 128
NCORES = 8
GROUPS = 4            # TP groups
DG = E // GROUPS      # 256 cols per core
ECH = E // P          # 8 contraction chunks
NCH = N // P          # 16 sequence chunks
QS = 1024             # q span for softmax tiles
QB = 512              # matmul moving free dim

_CACHE = {}


def _build():
    import sys
    if "/opt/trn_rl_repo" not in sys.path:
        sys.path.insert(0, "/opt/trn_rl_repo")
    import concourse.tile as tile
    from concourse import bacc, mybir
    from concourse.bass import ts

    F32 = mybir.dt.float32
    BF16 = mybir.dt.bfloat16
    Exp = mybir.ActivationFunctionType.Exp

    nc = bacc.Bacc("TRN2", target_bir_lowering=False, debug=False, num_devices=NCORES)

    xt = nc.dram_tensor("xt", [P, ECH, N], BF16, kind="ExternalInput").ap()
    wq = nc.dram_tensor("wq", [P, ECH, DG], BF16, kind="ExternalInput").ap()
    wk = nc.dram_tensor("wk", [P, ECH, DG], BF16, kind="ExternalInput").ap()
    wv = nc.dram_tensor("wv", [P, ECH, DG], BF16, kind="ExternalInput").ap()
    wo = nc.dram_tensor("wo", [P, 2, E], BF16, kind="ExternalInput").ap()
    bq2 = nc.dram_tensor("bq2", [P, 2], F32, kind="ExternalInput").ap()
    out = nc.dram_tensor("out", [N, E], BF16, kind="ExternalOutput").ap()

    with tile.TileContext(nc) as tc:
        with tc.tile_pool(name="persist", bufs=1) as pers, \
             tc.tile_pool(name="pexp", bufs=12) as pexp_pool, \
             tc.tile_pool(name="small", bufs=2) as small, \
             tc.tile_pool(name="ostage", bufs=4) as ostage, \
             tc.tile_pool(name="ppmain", bufs=1, space="PSUM") as ppm, \
             tc.tile_pool(name="ppoacc", bufs=1, space="PSUM") as ppo:
            xt_sb = pers.tile([P, ECH, N], BF16, tag="xt")
            wq_sb = pers.tile([P, ECH, DG], BF16, tag="wq")
            wk_sb = pers.tile([P, ECH, DG], BF16, tag="wk")
            wv_sb = pers.tile([P, ECH, DG], BF16, tag="wv")
            wo_sb = pers.tile([P, 2, E], BF16, tag="wo")
            bq_sb = pers.tile([P, 2], F32, tag="bq")
            qT_p = [pers.tile([P, N], BF16, tag=f"qT{i}", name=f"qT{i}") for i in range(2)]
            kT_p = [pers.tile([P, N], BF16, tag=f"kT{i}", name=f"kT{i}") for i in range(2)]
            v_sb = pers.tile([P, NCH, GROUPS, 66], BF16, tag="v")
            oT_p = [pers.tile([P, N], BF16, tag=f"oT{i}", name=f"oT{i}") for i in range(2)]

            def proj_ps(i, name):
                return ppm.tile([P, QS], F32, tag="A" if i % 2 == 0 else "B", name=name)

            def qk_chain(pair, w_sb, dst, bias, qb):
                # one 512-wide q block of the Q^T/K^T projection: 8 ec-chunk
                # matmuls accumulated in one PSUM bank, then drained on DVE
                def emit():
                    ps = proj_ps(qb, f"qkps{pair}{qb}")
                    psl = ps[:, :QB]
                    for ec in range(ECH):
                        nc.tensor.matmul(
                            psl,
                            w_sb[:, ec, ts(pair, P)],
                            xt_sb[:, ec, ts(qb, QB)],
                            start=(ec == 0), stop=(ec == ECH - 1),
                        )
                    if bias:
                        nc.vector.tensor_add(
                            dst[:, ts(qb, QB)], psl,
                            bq_sb[:, pair, None].to_broadcast((P, QB)),
                        )
                    else:
                        nc.vector.tensor_copy(dst[:, ts(qb, QB)], psl)
                return emit

            def v_chain(ncx):
                def emit():
                    ps = proj_ps(ncx, f"vps{ncx}")
                    psl = ps[:, :DG]
                    for ec in range(ECH):
                        nc.tensor.matmul(
                            psl,
                            xt_sb[:, ec, ts(ncx, P)],
                            wv_sb[:, ec, :],
                            start=(ec == 0), stop=(ec == ECH - 1),
                        )
                    nc.vector.tensor_copy(
                        v_sb[:, ncx, :, 0:64],
                        psl.rearrange("p (h d) -> p h d", d=D),
                    )
                return emit

            def wo_chain(ncx, drain=None, dma_eng=None):
                # out[ncx*128:(ncx+1)*128, :]: both 512-wide halves of E share
                # each oT stationary load; accumulate over the two d-chunks
                def emit():
                    ps = ppm.tile([P, QS], F32, tag="A" if ncx % 2 == 0 else "B",
                                  name=f"wops{ncx}")
                    for dc in range(2):
                        for fb in range(2):
                            nc.tensor.matmul(
                                ps[:, ts(fb, QB)],
                                oT_p[dc][:, ts(ncx, P)],
                                wo_sb[:, dc, ts(fb, QB)],
                                start=(dc == 0), stop=(dc == 1),
                            )
                    ot = ostage.tile([P, QS], BF16, tag="ot", name="ot")
                    (drain or nc.vector.tensor_copy)(ot, ps)
                    (dma_eng or nc.sync).dma_start(out[ts(ncx, P), :], ot)
                return emit

            def emit_attn(pair, qs, fills=(), finish_prev=None, finish_kc=2):
                # fills: dict kc -> list of filler emitters.  finish_prev:
                # the previous call's deferred normalize tail — emitted at a
                # kc whose neighborhood has no fill drains, so those drains
                # (which hand PSUM banks back to the S matmuls) never queue
                # behind it on the in-order DVE.  Returns this call's own
                # deferred tail.
                fills = dict(fills)
                if finish_prev is not None:
                    fills.setdefault(finish_kc, []).insert(0, finish_prev)
                oaccs = [ppo.tile([65, QS], F32, tag=f"O{h}", name=f"oacc{h}")
                         for h in range(2)]
                def emit_pv(kc, pes):
                    for h in range(2):
                        hh = 2 * pair + h
                        for qb in range(QS // QB):
                            nc.tensor.matmul(
                                oaccs[h][:, ts(qb, QB)],
                                v_sb[:, kc, hh, 0:65],
                                pes[h][:, ts(qb, QB)],
                                start=(kc == 0), stop=(kc == NCH - 1),
                            )

                # software-pipelined: kc's PVs are emitted after kc+1's S
                # matmuls so every PE instruction is ready when the in-order
                # engine reaches it — no wait-queue parking, no PE idle, and
                # the clock stays at the full p-state
                prev = None
                for kc in range(NCH):
                    for f in fills.pop(kc, ()):
                        f()
                    pes = []
                    for h in range(2):
                        # qb-inner: both q halves share the kT stationary, and
                        # exp(h) is dispatchable right after its own S pair
                        ps = ppm.tile([P, QS], F32, tag="AB"[h], name=f"spsum{h}")
                        psl = slice(D * h, D * h + D)
                        for qb in range(QS // QB):
                            nc.tensor.matmul(
                                ps[:, ts(qb, QB)],
                                kT_p[pair][psl, ts(kc, P)],
                                qT_p[pair][psl, qs * QS + qb * QB:qs * QS + (qb + 1) * QB],
                                start=True, stop=True,
                            )
                        pe = pexp_pool.tile([P, QS], BF16, tag="pexp", name="pexp")
                        nc.scalar.activation(pe, ps, Exp, scale=1.0 / 32.0)
                        pes.append(pe)
                    if prev is not None:
                        emit_pv(*prev)
                    prev = (kc, pes)
                emit_pv(*prev)
                for kc, fl in sorted(fills.items()):
                    for f in fl:
                        f()
                # free the oacc PSUM banks right away (the next call's first
                # PVs reuse them); the rest of the normalize is deferred
                osps = []
                for h in range(2):
                    osp = small.tile([65, QS], F32, tag=f"osp{h}", name="osp", bufs=1)
                    nc.vector.tensor_copy(osp, oaccs[h])
                    osps.append(osp)

                def finish():
                    for h in range(2):
                        psl = slice(D * h, D * h + D)
                        d2 = small.tile([1, QS], F32, tag=f"d2{h}", name="d2", bufs=1)
                        nc.vector.tensor_copy(d2, osps[h][64:65, :])
                        r2 = small.tile([1, QS], F32, tag=f"r2{h}", name="r2", bufs=1)
                        nc.vector.reciprocal_approx_fast(r2, d2)
                        rbc = small.tile([P, QS], F32, tag="rbc", name="rbc")
                        nc.gpsimd.partition_broadcast(rbc, r2)
                        nc.vector.tensor_mul(
                            oT_p[pair][psl, ts(qs, QS)],
                            osps[h][0:64, :],
                            rbc[0:64, :],
                        )
                return finish

            # --- DMA: one queue, ordered by first use — transfers only begin
            # ~8.4us in (runtime startup) and stripe across all engines at
            # ~400GB/s, so the queue order IS the arrival order
            nc.sync.dma_start(wk_sb, wk)
            nc.sync.dma_start(xt_sb[:, ts(0, 4), ts(0, QS)], xt[:, ts(0, 4), ts(0, QS)])
            nc.sync.dma_start(wq_sb, wq)
            # wv ahead of the second xt piece: the kc1 V-fills of call (0,0)
            # must never reach the head of the PE queue before wv has landed
            # (a not-yet-ready fill chain parks, stalls the PE, and drops it
            # out of the full p-state for the whole call)
            nc.sync.dma_start(wv_sb, wv)
            nc.sync.dma_start(xt_sb[:, ts(1, 4), ts(0, QS)], xt[:, ts(1, 4), ts(0, QS)])
            nc.sync.dma_start(bq_sb, bq2)
            nc.sync.dma_start(xt_sb[:, ts(0, 4), ts(1, QS)], xt[:, ts(0, 4), ts(1, QS)])
            nc.sync.dma_start(xt_sb[:, ts(1, 4), ts(1, QS)], xt[:, ts(1, 4), ts(1, QS)])
            nc.sync.dma_start(wo_sb, wo)

            ones_f32 = pers.tile([P, 1], F32, tag="ones")
            nc.vector.memset(ones_f32, 1.0)
            nc.vector.tensor_copy(
                v_sb[:, :, :, 64:65],
                ones_f32[:, 0, None, None, None].to_broadcast((P, NCH, GROUPS, 1)),
            )

            # prologue: K0/Q0 for q<1024 only (paced by the xt nh0 DMAs)
            for qb in range(2):
                qk_chain(0, wk_sb, kT_p[0], False, qb)()
            for qb in range(2):
                qk_chain(0, wq_sb, qT_p[0], True, qb)()

            # call (0,0): V paced 2/kc from kc1 (wv/xt surely landed — a
            # not-yet-ready fill would clog the 4-deep PE wait queue), then
            # the q>=1024 half of K0/Q0 (needed from call (0,1) kc0)
            fills00 = {kc + 1: [v_chain(2 * kc), v_chain(2 * kc + 1)] for kc in range(8)}
            # K-qb2/qb3 feed this very call's S at kc8/kc12 (kT cols are the
            # key chunks); Q-qb2/qb3 only feed call (0,1)
            fills00[8].append(qk_chain(0, wk_sb, kT_p[0], False, 2))
            fills00[9] = [qk_chain(0, wk_sb, kT_p[0], False, 3)]
            fills00[10] = [qk_chain(0, wq_sb, qT_p[0], True, 2)]
            fills00[11] = [qk_chain(0, wq_sb, qT_p[0], True, 3)]
            fin = emit_attn(0, 0, fills00)

            # call (0,1): K1/Q1 fillers, front-loaded (needed by calls (1,*))
            fills01 = {}
            for qb in range(4):
                fills01[2 * qb] = [qk_chain(1, wk_sb, kT_p[1], False, qb)]
                fills01[2 * qb + 1] = [qk_chain(1, wq_sb, qT_p[1], True, qb)]
            fin = emit_attn(0, 1, fills01, finish_prev=fin)

            fin = emit_attn(1, 0, finish_prev=fin)

            # call (1,1): output projection for q<1024, in the back half so
            # the (1,0) normalize (oT1 writes, deferred to kc2) has drained
            fills11 = {8 + i: [wo_chain(i)] for i in range(8)}
            fin = emit_attn(1, 1, fills11, finish_prev=fin)
            fin()

            # tail: output projection for q>=1024; stage copies on the (now
            # idle) Scalar engine so they don't queue behind the last
            # normalize on DVE, and alternate output DMAs across the SP and
            # Activation queues so the final drain runs at 2x
            for ncx in range(8, 16):
                wo_chain(ncx, drain=nc.scalar.copy,
                         dma_eng=nc.scalar if ncx % 2 else nc.sync)()

    nc.compile()
    return nc


def _get_nc():
    if "nc" not in _CACHE:
        _CACHE["nc"] = _build()
    return _CACHE["nc"]


def kernel(x, Wq, bq, Wk, bk, Wv, bv, Wo, bo, **run_kwargs):
    import sys
    if "/opt/trn_rl_repo" not in sys.path:
        sys.path.insert(0, "/opt/trn_rl_repo")
    import ml_dtypes
    from concourse.bass_utils import run_bass_kernel_spmd

    BF = ml_dtypes.bfloat16
    x = np.asarray(x, dtype=np.float32)
    Wq = np.asarray(Wq, dtype=np.float32)
    Wk = np.asarray(Wk, dtype=np.float32)
    Wv = np.asarray(Wv, dtype=np.float32)
    Wo = np.asarray(Wo, dtype=np.float32)
    bq = np.asarray(bq, dtype=np.float32)
    bv = np.asarray(bv, dtype=np.float32)
    bo = np.asarray(bo, dtype=np.float32)

    nc = _get_nc()

    def tile_rows(a, d0):
        # [d0*P, M] -> [P, d0, M]
        return np.ascontiguousarray(
            a.reshape(d0, P, -1).transpose(1, 0, 2)).astype(BF)

    in_maps = []
    xts = [tile_rows(np.ascontiguousarray(x[b].T), ECH) for b in range(B)]
    for c in range(NCORES):
        b, g = divmod(c, GROUPS)
        cols = slice(g * DG, (g + 1) * DG)
        in_maps.append({
            "xt": xts[b],
            "wq": tile_rows(Wq[:, cols], ECH),
            "wk": tile_rows(Wk[:, cols], ECH),
            "wv": tile_rows(Wv[:, cols], ECH),
            "wo": tile_rows(Wo[cols, :], 2),
            "bq2": np.ascontiguousarray(bq[cols].reshape(2, P).T),
        })

    try:
        res = run_bass_kernel_spmd(nc, in_maps, core_ids=list(range(NCORES)), **run_kwargs)
    except Exception:
        # device may be wedged from a prior run; reset the accelerator once
        try:
            import ctypes
            lib = ctypes.CDLL("/opt/axon/libaxon_pjrt.so")
            lib.axon_reset.restype = ctypes.c_int
            lib.axon_reset()
        except Exception:
            pass
        res = run_bass_kernel_spmd(nc, in_maps, core_ids=list(range(NCORES)), **run_kwargs)
    if run_kwargs:
        _CACHE["last_results"] = res

    # gather: sum TP partials per batch, add separable bias terms
    bias_vec = bv @ Wo + bo  # softmax rows sum to 1 => bv contributes bv@Wo
    full = np.empty((B, N, E), dtype=np.float32)
    for b in range(B):
        acc = res.results[b * GROUPS]["out"].astype(np.float32)
        for g in range(1, GROUPS):
            acc = acc + res.results[b * GROUPS + g]["out"].astype(np.float32)
        full[b] = acc + bias_vec[None, :]
    return full
